# revision 1
# baseline (speedup 1.0000x reference)
"""APPNP (nn_APPNP_3951369912454) on 8 trn2 NeuronCores via Bass.

h0 = MLP(X) on PE (bf16 matmuls, f32 accum); K=10 propagation hops
h_{k+1} = 0.9 * Dd^-1/2 A Do^-1/2 h_k + 0.1 * h0.

Key ideas:
- rsqrt(deg) edge coeffs are separable -> folded into per-node scalings
  (table rows pre-scaled by norm_src, aggregates scaled by norm_dst).
- nodes renumbered by in-degree and striped across 8 shards; each core
  owns packed ids [c*12544, c*12544+12500), 44 pad ids stay zero rows.
- per hop, each core gathers h rows of its in-edges from a replicated
  [100352, 64] f32 DRAM table via gpsimd.dma_gather. The Q7 ucode
  multiplies indices SIGNED (IVP_MULUSAN_2X32), so with the gather base
  at window_start+32768 rows an int16 index spans a 65536-row window =>
  only 2 overlapping windows. Each window uses its own
  window-degree-sorted dst packing so per-column padded degrees are
  tight; every call ends with a positive pad slot so the Q7
  trailing-negative trim never drops a real edge.
- segment sums = fixed-structure strided tensor_reduce.
- window outputs merged by 2x2 small combine-gathers (window->packed
  permutation absorbed into gather indices).
- per-hop AllGather replicates each core's new shard into the table.
"""
import sys

sys.path.insert(0, "/opt/trn_rl_repo")

import numpy as np

import concourse.bass as bass
import concourse.bacc as bacc
import concourse.mybir as mybir
import concourse.tile as tile
from concourse.masks import make_identity

NCORES = 8
N = 100000
F_IN = 512
F_HID = 256
F_OUT = 64
K_HOPS = 10
ALPHA = 0.1

# experiment flags (A/B attribution; defaults = production behavior)
SKIP_GATHER = False
SKIP_REDUCE = False
SKIP_ALLGATHER = False
SKIP_W0 = False
SKIP_COMBINE = False
SINGLE_PACKET = False
SCRATCH = 16384

SH = 12544              # shard width (12500 real + 44 zero pad) = 98*128
D = 98                  # dst columns per partition
T_ROWS = NCORES * SH    # 100352
WIN = 65536             # int16 read as SIGNED by Q7 addr math -> mid-base trick
WIN_BASES = (0, T_ROWS - WIN)
NW = 2
CHUNK_SLOTS = 63        # max gather slots/partition per call (+1 tail pad)
NQ = 4                  # SWDGE queues


def _wrap16(flat):
    """flat [n] int16 -> [16, n//16] wrapped (one partition group; the
    device replicates to all 8 groups once at kernel start)."""
    n = flat.shape[0]
    return np.ascontiguousarray(flat.reshape(n // 16, 16).T)


def preprocess(features, edge_index, W1, b1, W2, b2):
    src = np.asarray(edge_index[0], dtype=np.int64)
    dst = np.asarray(edge_index[1], dtype=np.int64)
    deg_out = np.bincount(src, minlength=N)
    deg_in = np.bincount(dst, minlength=N)

    order = np.argsort(deg_in, kind="stable")          # rank -> old id
    new_id = np.empty(N, np.int64)
    new_id[order] = (np.arange(N) % NCORES) * SH + np.arange(N) // NCORES

    # pass 2: re-sort each shard's nodes by their window-0 in-degree so the
    # window-0 packing is the IDENTITY (its reduce writes agg directly; no
    # combine un-permute needed for w0).  Window membership of shard-5 srcs
    # shifts slightly after the re-sort; packing uses exact recounts so only
    # padding tightness is (mildly) affected.
    w_of_src0 = (new_id[src] >= WIN).astype(np.int64)
    cnt_w0 = np.zeros(N, np.int64)
    np.add.at(cnt_w0, dst, 1 - w_of_src0)
    shard_of = new_id // SH
    for c in range(NCORES):
        olds = np.where(shard_of == c)[0]              # old ids in shard c
        o2 = olds[np.argsort(cnt_w0[olds], kind="stable")]
        new_id[o2] = c * SH + np.arange(len(o2))

    ns = (1.0 / np.sqrt(np.maximum(deg_out, 1.0))).astype(np.float64)
    nd = (1.0 / np.sqrt(np.maximum(deg_in, 1.0))).astype(np.float64)
    ns_new = np.zeros(T_ROWS); nd_new = np.zeros(T_ROWS)
    ns_new[new_id] = ns
    nd_new[new_id] = nd

    src_n = new_id[src]
    dst_n = new_id[dst]
    w_of_src = (src_n >= WIN).astype(np.int64)     # first containing window

    # zero rows in the POSITIVE int16 half of each window (rel >= 32768),
    # so a trailing pad slot never triggers the Q7 trailing-negative trim.
    zid = []
    for b in WIN_BASES:
        z = None
        for s in range(NCORES):
            cand = s * SH + 12500
            if b + 32768 <= cand < b + WIN:
                z = cand; break
        assert z is not None, b
        zid.append(z)

    counts = np.zeros((NCORES, NW, SH), np.int64)
    core_of = dst_n // SH
    q_of = dst_n % SH
    np.add.at(counts, (core_of, w_of_src, q_of), 1)

    pos_w = np.empty((NCORES, NW, SH), np.int64)
    for c in range(NCORES):
        pos_w[c, 0] = np.arange(SH)                    # w0: identity
        o = np.argsort(counts[c, 1], kind="stable")
        pos_w[c, 1, o] = np.arange(SH)

    packed_counts = np.empty_like(counts)
    for c in range(NCORES):
        for w in range(NW):
            packed_counts[c, w, pos_w[c, w]] = counts[c, w]
    colmax = packed_counts.reshape(NCORES, NW, D, 128).max(axis=3)
    P = np.maximum(colmax.max(axis=0), 1)              # [NW, D]
    assert P.max() <= CHUNK_SLOTS, f"P max {P.max()} > chunk"

    # chunk list (shared across cores/hops): (w, j0, j1, tot, g16)
    # tot includes one trailing all-pad slot column (positive idx) so the
    # Q7 trailing-negative trim never eats a real edge.
    chunks_all = []
    g16 = 0
    KW = []
    for w in range(NW):
        j = 0; kw = 0
        while j < D:
            j0 = j; tot = 0
            while j < D and tot + P[w, j] <= CHUNK_SLOTS:
                tot += P[w, j]; j += 1
            tot += 1                # trailing pad slot column
            chunks_all.append((w, j0, j, int(tot), g16))
            g16 += tot * 8          # tot*128 idx / 16
            kw += tot
        KW.append(kw)
    GW = g16
    slots_total = sum(KW) * 128
    print(f"[preprocess] slots/core/hop = {slots_total} "
          f"(edges/core ~{len(src)//NCORES}, overhead "
          f"{slots_total/(len(src)/NCORES)-1:+.1%}), KW={KW}")

    Xp = np.zeros((NCORES, SH, F_IN), np.float32)
    feats = np.asarray(features, dtype=np.float32)
    inv_rows = np.argsort(new_id, kind="stable")   # old ids in ascending-row order
    for c in range(NCORES):
        ids = inv_rows[c * 12500:(c + 1) * 12500]
        Xp[c, :len(ids)] = feats[ids]

    ekey = (core_of * NW + w_of_src) * SH + pos_w[core_of, w_of_src, q_of]
    eorder = np.argsort(ekey, kind="stable")
    src_s = src_n[eorder]
    cw_s = (core_of * NW + w_of_src)[eorder]
    posd_s = pos_w[core_of, w_of_src, q_of][eorder]
    cw_tot = np.zeros(NCORES * NW + 1, np.int64)
    np.add.at(cw_tot, cw_s + 1, 1)
    cw_tot = np.cumsum(cw_tot)

    col_off = np.zeros((NW, D), np.int64)
    for w in range(NW):
        acc = 0
        for j in range(D):
            col_off[w, j] = acc
            acc += P[w, j]

    core_inputs = []
    for c in range(NCORES):
        gparts = []
        for w in range(NW):
            lo, hi = cw_tot[c * NW + w], cw_tot[c * NW + w + 1]
            es = src_s[lo:hi]
            ep = posd_s[lo:hi]
            zrel = zid[w] - WIN_BASES[w] - 32768       # positive int16
            slot = np.full((128, KW[w] + 64), zrel, np.int32)
            pcol = (ep % 128).astype(np.int64)
            jcol = (ep // 128).astype(np.int64)
            grp = pcol * D + jcol
            o2 = np.argsort(grp, kind="stable")
            es2 = es[o2]; g2 = grp[o2]
            seq = np.arange(len(g2)) - np.searchsorted(g2, g2, side="left")
            slot[g2 // D, col_off[w, g2 % D] + seq] = \
                es2 - WIN_BASES[w] - 32768              # signed rel idx
            for (ww, j0, j1, tot, _g) in chunks_all:
                if ww != w:
                    continue
                sub = np.full((128, tot), zrel, np.int32)
                real = tot - 1                          # minus tail pad col
                sub[:, :real] = slot[:, col_off[w, j0]:col_off[w, j0] + real]
                flat = sub.T.ravel()                    # i = slot*128 + p
                assert flat[-1] >= 0
                gparts.append(_wrap16(flat.astype(np.int16)))
        gidx = np.concatenate(gparts, axis=1)
        assert gidx.shape[1] == GW

        cparts = []
        q_grid = np.arange(SH)                          # i = j*128+p == q
        rows = pos_w[c, 1][q_grid]                      # only w1 un-permutes
        half = SH // 2
        cparts.append(_wrap16(rows[:half].astype(np.int16)))
        cparts.append(_wrap16(rows[half:].astype(np.int16)))
        cidx = np.concatenate(cparts, axis=1)

        sl = slice(c * SH, (c + 1) * SH)
        bf16 = mybir.dt.np(mybir.dt.bfloat16)
        core_inputs.append({
            "xt": np.ascontiguousarray(Xp[c].T).astype(bf16),
            "w1": np.asarray(W1, np.float32),
            "b1": np.asarray(b1, np.float32).reshape(F_HID, 1),
            "w2": np.asarray(W2, np.float32),
            "b2": np.asarray(b2, np.float32).reshape(F_OUT, 1),
            "nd09": (0.9 * nd_new[sl]).astype(np.float32).reshape(D, 128).T.copy(),
            "ns1": ns_new[sl].astype(np.float32).reshape(D, 128).T.copy(),
            "gidx": gidx,
            "cidx": cidx,
        })

    struct = {"P": P, "chunks": chunks_all, "KW": KW, "GW": GW}
    inv = np.argsort(new_id, kind="stable")    # old ids in ascending-row order
    ids = [inv[c * 12500:(c + 1) * 12500] for c in range(NCORES)]
    perm = {"ids": ids}
    return core_inputs, struct, perm


def build_nc(struct):
    P = struct["P"]
    chunks_all = struct["chunks"]
    GW = struct["GW"]
    fdt = mybir.dt.float32
    bdt = mybir.dt.bfloat16

    CW = 2 * (SH // 2) // 16
    nc = bacc.Bacc("TRN2", num_devices=NCORES, num_swdge_queues=NQ,
                   dynamic_dma_scratch_size=SCRATCH)
    xt = nc.dram_tensor("xt", [F_IN, SH], bdt, kind="ExternalInput")
    w1 = nc.dram_tensor("w1", [F_IN, F_HID], fdt, kind="ExternalInput")
    b1 = nc.dram_tensor("b1", [F_HID, 1], fdt, kind="ExternalInput")
    w2 = nc.dram_tensor("w2", [F_HID, F_OUT], fdt, kind="ExternalInput")
    b2 = nc.dram_tensor("b2", [F_OUT, 1], fdt, kind="ExternalInput")
    nd09 = nc.dram_tensor("nd09", [128, D], fdt, kind="ExternalInput")
    ns1 = nc.dram_tensor("ns1", [128, D], fdt, kind="ExternalInput")
    gidx16 = nc.dram_tensor("gidx", [16, GW], mybir.dt.int16,
                            kind="ExternalInput")
    cidx16 = nc.dram_tensor("cidx", [16, CW], mybir.dt.int16,
                            kind="ExternalInput")
    hout = nc.dram_tensor("hout", [SH, F_OUT], fdt, kind="ExternalOutput")

    with tile.TileContext(nc) as tc:
        with tc.tile_pool(name="dram", bufs=1, space="DRAM") as dpool, \
             tc.tile_pool(name="big", bufs=1) as big, \
             tc.tile_pool(name="psum", bufs=2, space="PSUM") as pp:

            # bf16 zero-padded table rows: [h(64)|0(64)] = 256B per row
            tabA = dpool.tile([T_ROWS, 2 * F_OUT], bdt)
            tabB = dpool.tile([T_ROWS, 2 * F_OUT], bdt)
            stag = dpool.tile([D, 128, 2 * F_OUT], bdt)
            outw = [dpool.tile([SH, F_OUT], fdt, tag="outw1", name="outw1")]
            # replicate compact idx uploads to all 8 partition groups
            gidx = dpool.tile([128, GW], mybir.dt.int16)
            cidx = dpool.tile([128, CW], mybir.dt.int16)
            for g in range(8):
                nc.sync.dma_start(gidx[16 * g:16 * (g + 1), :], gidx16[:])
                nc.sync.dma_start(cidx[16 * g:16 * (g + 1), :], cidx16[:])

            h0a = big.tile([128, D, F_OUT], fdt)
            hcur = big.tile([128, D, F_OUT], fdt)
            agg = big.tile([128, D, F_OUT], fdt)
            sclb = big.tile([128, D, 2 * F_OUT], bdt)
            nc.vector.memset(sclb[:], 0.0)
            nd_sb = big.tile([128, D], fdt)
            ns_sb = big.tile([128, D], fdt)
            nc.sync.dma_start(nd_sb[:], nd09[:])
            nc.sync.dma_start(ns_sb[:], ns1[:])
            ident = big.tile([64, 64], fdt)
            make_identity(nc, ident[:])

            # ------------------------------------------------ MLP -> h0a
            w1b = big.tile([128, 4, F_HID], bdt)
            for kc in range(4):
                nc.gpsimd.dma_start(w1b[:, kc, :], w1[kc * 128:(kc + 1) * 128, :])
            w2b = big.tile([128, 2, F_OUT], bdt)
            for mh in range(2):
                nc.gpsimd.dma_start(w2b[:, mh, :], w2[mh * 128:(mh + 1) * 128, :])
            b1s = big.tile([128, 2], fdt)
            nc.sync.dma_start(b1s[:, 0:1], b1[0:128, :])
            nc.sync.dma_start(b1s[:, 1:2], b1[128:256, :])
            b2s = big.tile([64, 1], fdt)
            nc.sync.dma_start(b2s[:, :], b2[:, :])

            NT = 512
            tiles = [(i * NT, NT) for i in range(SH // NT)]
            if SH % NT:
                tiles.append((SH - SH % NT, SH % NT))
            with tc.tile_pool(name="mlp", bufs=3) as mp:
                for (r0, nr) in tiles:
                    xtb = mp.tile([128, 4, NT], bdt, tag="xtb")
                    for kc in range(4):
                        nc.sync.dma_start(
                            xtb[:, kc, :nr],
                            xt[kc * 128:(kc + 1) * 128, r0:r0 + nr])
                    h1 = mp.tile([128, 2, NT], bdt, tag="h1")
                    for mh in range(2):
                        ps = pp.tile([128, NT], fdt, tag="ps1")
                        for kc in range(4):
                            nc.tensor.matmul(
                                ps[:, :nr],
                                lhsT=w1b[:, kc, mh * 128:(mh + 1) * 128],
                                rhs=xtb[:, kc, :nr],
                                start=(kc == 0), stop=(kc == 3))
                        nc.scalar.activation(h1[:, mh, :nr], ps[:, :nr],
                                             mybir.ActivationFunctionType.Relu,
                                             bias=b1s[:, mh:mh + 1])
                    ps2 = pp.tile([64, NT], fdt, tag="ps2")
                    for mh in range(2):
                        nc.tensor.matmul(ps2[:, :nr], lhsT=w2b[:, mh, :],
                                         rhs=h1[:, mh, :nr],
                                         start=(mh == 0), stop=(mh == 1))
                    h2 = mp.tile([64, NT], fdt, tag="h2")
                    nc.vector.tensor_scalar_add(h2[:, :nr], ps2[:, :nr],
                                                b2s[:, :])
                    for s in range(nr // 128):
                        pst = pp.tile([128, 64], fdt, tag="pst")
                        nc.tensor.transpose(pst[:], h2[:, s * 128:(s + 1) * 128],
                                            ident[:])
                        j = (r0 + s * 128) // 128
                        nc.scalar.activation(h0a[:, j, :], pst[:],
                                             mybir.ActivationFunctionType.Copy,
                                             scale=ALPHA)
            nc.vector.tensor_scalar_mul(hcur[:], h0a[:], 1.0 / ALPHA)

            # ------------------------------------------------ hops
            with tc.tile_pool(name="hop", bufs=1) as hp, \
                 tc.tile_pool(name="wk", bufs=3) as wk, \
                 tc.tile_pool(name="cgp", bufs=2) as cgp:
                for hop in range(K_HOPS):
                    tab = tabA if hop % 2 == 0 else tabB
                    nc.vector.tensor_tensor(
                        out=sclb[:, :, :F_OUT], in0=hcur[:],
                        in1=ns_sb[:].unsqueeze(-1).to_broadcast([128, D, F_OUT]),
                        op=mybir.AluOpType.mult)
                    nc.sync.dma_start(stag[:].rearrange("j p f -> p j f"),
                                      sclb[:])
                    if not SKIP_ALLGATHER:
                        nc.gpsimd.collective_compute(
                            "AllGather", mybir.AluOpType.bypass,
                            replica_groups=[list(range(NCORES))],
                            ins=[stag[:].rearrange("j p f -> (j p) f")],
                            outs=[tab[:]])

                    qn = 0
                    for w in range(NW):
                        # w0 packing is identity: reduce straight into agg.
                        ow = agg if w == 0 else hp.tile([128, D, F_OUT], fdt,
                                                        tag="ow")
                        if SKIP_GATHER or SKIP_REDUCE or (SKIP_W0 and w == 0):
                            nc.vector.memset(ow[:], 0.0)
                        if SKIP_W0 and w == 0:
                            continue
                        for (ww, j0, j1, tot, g16) in chunks_all:
                            if ww != w:
                                continue
                            nidx = tot * 128
                            it = wk.tile([128, (CHUNK_SLOTS + 1) * 8], mybir.dt.int16,
                                         tag="gi")
                            nc.sync.dma_start(it[:, :nidx // 16],
                                              gidx[:, g16:g16 + nidx // 16])
                            msg = wk.tile([128, CHUNK_SLOTS + 1, 2 * F_OUT],
                                          bdt, tag="msg")
                            if not SKIP_GATHER:
                                nc.gpsimd.dma_gather(
                                    msg[:, :tot, :],
                                    tab[WIN_BASES[w] + 32768:
                                        WIN_BASES[w] + 32768 + 32768, :],
                                    it[:, :nidx // 16], nidx, nidx, 2 * F_OUT,
                                    single_packet=SINGLE_PACKET,
                                    queue_num=qn % NQ)
                            qn += 1
                            soff = 0
                            j = j0
                            while j < j1:
                                je = j
                                while je < j1 and P[w, je] == P[w, j]:
                                    je += 1
                                pc = int(P[w, j]); ncol = je - j
                                seg = msg[:, soff:soff + ncol * pc, :F_OUT
                                          ].rearrange(
                                    "p (c q) f -> p c f q", q=pc)
                                if not (SKIP_REDUCE or SKIP_GATHER):
                                    nc.vector.tensor_reduce(
                                        out=ow[:, j:je, :], in_=seg,
                                        axis=mybir.AxisListType.X,
                                        op=mybir.AluOpType.add)
                                soff += ncol * pc
                                j = je
                        if w == 1:
                            nc.sync.dma_start(
                                outw[0][:].rearrange("(v p) f -> p v f", p=128),
                                ow[:])

                    half = SH // 2
                    ch16 = half // 16
                    for hh in range(2):
                        if SKIP_COMBINE:
                            continue
                        cit = wk.tile([128, ch16], mybir.dt.int16, tag="ci")
                        off = hh * ch16
                        nc.sync.dma_start(cit[:], cidx[:, off:off + ch16])
                        cg = cgp.tile([128, D // 2, F_OUT], fdt, tag="cg")
                        nc.gpsimd.dma_gather(
                            cg[:], outw[0][:], cit[:], half, half, F_OUT,
                            single_packet=False, queue_num=qn % NQ)
                        qn += 1
                        dstslc = agg[:, hh * (D // 2):(hh + 1) * (D // 2), :]
                        nc.vector.tensor_add(dstslc, dstslc, cg[:])

                    nc.vector.tensor_tensor(
                        out=hcur[:], in0=agg[:],
                        in1=nd_sb[:].unsqueeze(-1).to_broadcast([128, D, F_OUT]),
                        op=mybir.AluOpType.mult)
                    nc.vector.tensor_add(hcur[:], hcur[:], h0a[:])

            nc.sync.dma_start(hout[:].rearrange("(j p) f -> p j f", p=128),
                              hcur[:])
    nc.finalize()
    return nc


def kernel(features, edge_index, W1, b1, W2, b2):
    core_inputs, struct, perm = preprocess(features, edge_index, W1, b1, W2, b2)
    nc = build_nc(struct)
    from concourse.bass_utils import run_bass_kernel_spmd
    res = run_bass_kernel_spmd(nc, core_inputs, core_ids=list(range(NCORES)))
    out = np.zeros((N, F_OUT), np.float32)
    for c in range(NCORES):
        h = res.results[c]["hout"]                     # [SH, 64] row order
        ids = perm["ids"][c]                           # old ids in row order
        out[ids] = h[:len(ids)]
    return out



# revision 2
# speedup vs baseline: 3.0445x; 3.0445x over previous
"""APPNP (nn_APPNP_3951369912454) on 8 trn2 NeuronCores via Bass.

h0 = MLP(X) on PE (bf16 matmuls, f32 accum); K=10 propagation hops
h_{k+1} = 0.9 * Dd^-1/2 A Do^-1/2 h_k + 0.1 * h0.

Key ideas:
- rsqrt(deg) edge coeffs are separable -> folded into per-node scalings
  (table rows pre-scaled by norm_src, aggregates scaled by norm_dst).
- nodes renumbered by in-degree and striped across 8 shards; each core
  owns packed ids [c*12544, c*12544+12500), 44 pad ids stay zero rows.
- per hop, each core gathers h rows of its in-edges from a replicated
  [100352, 64] f32 DRAM table via gpsimd.dma_gather. The Q7 ucode
  multiplies indices SIGNED (IVP_MULUSAN_2X32), so with the gather base
  at window_start+32768 rows an int16 index spans a 65536-row window =>
  only 2 overlapping windows. Each window uses its own
  window-degree-sorted dst packing so per-column padded degrees are
  tight; every call ends with a positive pad slot so the Q7
  trailing-negative trim never drops a real edge.
- segment sums = fixed-structure strided tensor_reduce.
- window outputs merged by 2x2 small combine-gathers (window->packed
  permutation absorbed into gather indices).
- per-hop AllGather replicates each core's new shard into the table.
"""
import sys

sys.path.insert(0, "/opt/trn_rl_repo")

import numpy as np

import concourse.bass as bass
import concourse.bacc as bacc
import concourse.mybir as mybir
import concourse.tile as tile
from concourse.masks import make_identity

NCORES = 8
N = 100000
F_IN = 512
F_HID = 256
F_OUT = 64
K_HOPS = 10
ALPHA = 0.1

# experiment flags (A/B attribution; defaults = production behavior)
SKIP_GATHER = False
SKIP_REDUCE = False
SKIP_ALLGATHER = False
SKIP_W0 = False
SKIP_COMBINE = False
SINGLE_PACKET = False
SCRATCH = 16384
SHARED_TAB = True       # AllGather output in Shared DRAM (write-once fast path)
FLAT_AG = False         # flatten collective APs to 1D
WK_BUFS = 4             # gather work-tile ring depth (4+ keeps all queues busy)

SH = 12544              # shard width (12500 real + 44 zero pad) = 98*128
D = 98                  # dst columns per partition
T_ROWS = NCORES * SH    # 100352
WIN = 65536             # int16 read as SIGNED by Q7 addr math -> mid-base trick
WIN_BASES = (0, T_ROWS - WIN)
NW = 2
CHUNK_SLOTS = 63        # max gather slots/partition per call (+1 tail pad)
NQ = 4                  # SWDGE queues


def _wrap16(flat):
    """flat [n] int16 -> [16, n//16] wrapped (one partition group; the
    device replicates to all 8 groups once at kernel start)."""
    n = flat.shape[0]
    return np.ascontiguousarray(flat.reshape(n // 16, 16).T)


def preprocess(features, edge_index, W1, b1, W2, b2):
    src = np.asarray(edge_index[0], dtype=np.int64)
    dst = np.asarray(edge_index[1], dtype=np.int64)
    deg_out = np.bincount(src, minlength=N)
    deg_in = np.bincount(dst, minlength=N)

    order = np.argsort(deg_in, kind="stable")          # rank -> old id
    new_id = np.empty(N, np.int64)
    new_id[order] = (np.arange(N) % NCORES) * SH + np.arange(N) // NCORES

    # pass 2: re-sort each shard's nodes by their window-0 in-degree so the
    # window-0 packing is the IDENTITY (its reduce writes agg directly; no
    # combine un-permute needed for w0).  Window membership of shard-5 srcs
    # shifts slightly after the re-sort; packing uses exact recounts so only
    # padding tightness is (mildly) affected.
    w_of_src0 = (new_id[src] >= WIN).astype(np.int64)
    cnt_w0 = np.zeros(N, np.int64)
    np.add.at(cnt_w0, dst, 1 - w_of_src0)
    shard_of = new_id // SH
    for c in range(NCORES):
        olds = np.where(shard_of == c)[0]              # old ids in shard c
        o2 = olds[np.argsort(cnt_w0[olds], kind="stable")]
        new_id[o2] = c * SH + np.arange(len(o2))

    ns = (1.0 / np.sqrt(np.maximum(deg_out, 1.0))).astype(np.float64)
    nd = (1.0 / np.sqrt(np.maximum(deg_in, 1.0))).astype(np.float64)
    ns_new = np.zeros(T_ROWS); nd_new = np.zeros(T_ROWS)
    ns_new[new_id] = ns
    nd_new[new_id] = nd

    src_n = new_id[src]
    dst_n = new_id[dst]
    w_of_src = (src_n >= WIN).astype(np.int64)     # first containing window

    # zero rows in the POSITIVE int16 half of each window (rel >= 32768),
    # so a trailing pad slot never triggers the Q7 trailing-negative trim.
    zid = []
    for b in WIN_BASES:
        z = None
        for s in range(NCORES):
            cand = s * SH + 12500
            if b + 32768 <= cand < b + WIN:
                z = cand; break
        assert z is not None, b
        zid.append(z)

    counts = np.zeros((NCORES, NW, SH), np.int64)
    core_of = dst_n // SH
    q_of = dst_n % SH
    np.add.at(counts, (core_of, w_of_src, q_of), 1)

    pos_w = np.empty((NCORES, NW, SH), np.int64)
    for c in range(NCORES):
        pos_w[c, 0] = np.arange(SH)                    # w0: identity
        o = np.argsort(counts[c, 1], kind="stable")
        pos_w[c, 1, o] = np.arange(SH)

    packed_counts = np.empty_like(counts)
    for c in range(NCORES):
        for w in range(NW):
            packed_counts[c, w, pos_w[c, w]] = counts[c, w]
    colmax = packed_counts.reshape(NCORES, NW, D, 128).max(axis=3)
    P = np.maximum(colmax.max(axis=0), 1)              # [NW, D]
    assert P.max() <= CHUNK_SLOTS, f"P max {P.max()} > chunk"

    # chunk list (shared across cores/hops): (w, j0, j1, tot, g16)
    # tot includes one trailing all-pad slot column (positive idx) so the
    # Q7 trailing-negative trim never eats a real edge.
    chunks_all = []
    g16 = 0
    KW = []
    for w in range(NW):
        j = 0; kw = 0
        while j < D:
            j0 = j; tot = 0
            while j < D and tot + P[w, j] <= CHUNK_SLOTS:
                tot += P[w, j]; j += 1
            tot += 1                # trailing pad slot column
            chunks_all.append((w, j0, j, int(tot), g16))
            g16 += tot * 8          # tot*128 idx / 16
            kw += tot
        KW.append(kw)
    GW = g16
    slots_total = sum(KW) * 128
    print(f"[preprocess] slots/core/hop = {slots_total} "
          f"(edges/core ~{len(src)//NCORES}, overhead "
          f"{slots_total/(len(src)/NCORES)-1:+.1%}), KW={KW}")

    Xp = np.zeros((NCORES, SH, F_IN), np.float32)
    feats = np.asarray(features, dtype=np.float32)
    inv_rows = np.argsort(new_id, kind="stable")   # old ids in ascending-row order
    for c in range(NCORES):
        ids = inv_rows[c * 12500:(c + 1) * 12500]
        Xp[c, :len(ids)] = feats[ids]

    ekey = (core_of * NW + w_of_src) * SH + pos_w[core_of, w_of_src, q_of]
    eorder = np.argsort(ekey, kind="stable")
    src_s = src_n[eorder]
    cw_s = (core_of * NW + w_of_src)[eorder]
    posd_s = pos_w[core_of, w_of_src, q_of][eorder]
    cw_tot = np.zeros(NCORES * NW + 1, np.int64)
    np.add.at(cw_tot, cw_s + 1, 1)
    cw_tot = np.cumsum(cw_tot)

    col_off = np.zeros((NW, D), np.int64)
    for w in range(NW):
        acc = 0
        for j in range(D):
            col_off[w, j] = acc
            acc += P[w, j]

    core_inputs = []
    for c in range(NCORES):
        gparts = []
        for w in range(NW):
            lo, hi = cw_tot[c * NW + w], cw_tot[c * NW + w + 1]
            es = src_s[lo:hi]
            ep = posd_s[lo:hi]
            zrel = zid[w] - WIN_BASES[w] - 32768       # positive int16
            slot = np.full((128, KW[w] + 64), zrel, np.int32)
            pcol = (ep % 128).astype(np.int64)
            jcol = (ep // 128).astype(np.int64)
            grp = pcol * D + jcol
            o2 = np.argsort(grp, kind="stable")
            es2 = es[o2]; g2 = grp[o2]
            seq = np.arange(len(g2)) - np.searchsorted(g2, g2, side="left")
            slot[g2 // D, col_off[w, g2 % D] + seq] = \
                es2 - WIN_BASES[w] - 32768              # signed rel idx
            for (ww, j0, j1, tot, _g) in chunks_all:
                if ww != w:
                    continue
                sub = np.full((128, tot), zrel, np.int32)
                real = tot - 1                          # minus tail pad col
                sub[:, :real] = slot[:, col_off[w, j0]:col_off[w, j0] + real]
                flat = sub.T.ravel()                    # i = slot*128 + p
                assert flat[-1] >= 0
                gparts.append(_wrap16(flat.astype(np.int16)))
        gidx = np.concatenate(gparts, axis=1)
        assert gidx.shape[1] == GW

        cparts = []
        q_grid = np.arange(SH)                          # i = j*128+p == q
        rows = pos_w[c, 1][q_grid]                      # only w1 un-permutes
        half = SH // 2
        cparts.append(_wrap16(rows[:half].astype(np.int16)))
        cparts.append(_wrap16(rows[half:].astype(np.int16)))
        cidx = np.concatenate(cparts, axis=1)

        sl = slice(c * SH, (c + 1) * SH)
        bf16 = mybir.dt.np(mybir.dt.bfloat16)
        core_inputs.append({
            "xt": np.ascontiguousarray(Xp[c].T).astype(bf16),
            "w1": np.asarray(W1, np.float32),
            "b1": np.asarray(b1, np.float32).reshape(F_HID, 1),
            "w2": np.asarray(W2, np.float32),
            "b2": np.asarray(b2, np.float32).reshape(F_OUT, 1),
            "nd09": (0.9 * nd_new[sl]).astype(np.float32).reshape(D, 128).T.copy(),
            "ns1": ns_new[sl].astype(np.float32).reshape(D, 128).T.copy(),
            "gidx": gidx,
            "cidx": cidx,
        })

    struct = {"P": P, "chunks": chunks_all, "KW": KW, "GW": GW}
    inv = np.argsort(new_id, kind="stable")    # old ids in ascending-row order
    ids = [inv[c * 12500:(c + 1) * 12500] for c in range(NCORES)]
    perm = {"ids": ids}
    return core_inputs, struct, perm


def build_nc(struct):
    P = struct["P"]
    chunks_all = struct["chunks"]
    GW = struct["GW"]
    fdt = mybir.dt.float32
    bdt = mybir.dt.bfloat16

    CW = 2 * (SH // 2) // 16
    nc = bacc.Bacc("TRN2", num_devices=NCORES, num_swdge_queues=NQ,
                   dynamic_dma_scratch_size=SCRATCH)
    xt = nc.dram_tensor("xt", [F_IN, SH], bdt, kind="ExternalInput")
    w1 = nc.dram_tensor("w1", [F_IN, F_HID], fdt, kind="ExternalInput")
    b1 = nc.dram_tensor("b1", [F_HID, 1], fdt, kind="ExternalInput")
    w2 = nc.dram_tensor("w2", [F_HID, F_OUT], fdt, kind="ExternalInput")
    b2 = nc.dram_tensor("b2", [F_OUT, 1], fdt, kind="ExternalInput")
    nd09 = nc.dram_tensor("nd09", [128, D], fdt, kind="ExternalInput")
    ns1 = nc.dram_tensor("ns1", [128, D], fdt, kind="ExternalInput")
    gidx16 = nc.dram_tensor("gidx", [16, GW], mybir.dt.int16,
                            kind="ExternalInput")
    cidx16 = nc.dram_tensor("cidx", [16, CW], mybir.dt.int16,
                            kind="ExternalInput")
    hout = nc.dram_tensor("hout", [SH, F_OUT], fdt, kind="ExternalOutput")

    with tile.TileContext(nc) as tc:
        with tc.tile_pool(name="dram", bufs=1, space="DRAM") as dpool, \
             tc.tile_pool(name="big", bufs=1) as big, \
             tc.tile_pool(name="psum", bufs=2, space="PSUM") as pp:

            # bf16 zero-padded table rows: [h(64)|0(64)] = 256B per row.
            # one table per hop: Shared tensors allow a single writer inst.
            tab_space = "Shared" if SHARED_TAB else "Local"
            tabs = [dpool.tile([T_ROWS, 2 * F_OUT], bdt, addr_space=tab_space,
                               name=f"tab{h}", tag=f"tab{h}")
                    for h in range(K_HOPS)]
            stag = dpool.tile([D, 128, 2 * F_OUT], bdt)
            outw = [dpool.tile([SH, F_OUT], fdt, tag="outw1", name="outw1")]
            # replicate compact idx uploads to all 8 partition groups
            gidx = dpool.tile([128, GW], mybir.dt.int16)
            cidx = dpool.tile([128, CW], mybir.dt.int16)
            for g in range(8):
                nc.sync.dma_start(gidx[16 * g:16 * (g + 1), :], gidx16[:])
                nc.sync.dma_start(cidx[16 * g:16 * (g + 1), :], cidx16[:])

            h0a = big.tile([128, D, F_OUT], fdt)
            hcur = big.tile([128, D, F_OUT], fdt)
            agg = big.tile([128, D, F_OUT], fdt)
            sclb = big.tile([128, D, F_OUT], bdt)
            nc.vector.memset(sclb[:], 0.0)
            # stag's zero half is written once; per-hop DMA only rewrites
            # the h half (saves 12KB/partition of SBUF for deeper wk ring)
            nc.sync.dma_start(
                stag[:, :, F_OUT:2 * F_OUT].rearrange("j p f -> p j f"),
                sclb[:])
            nd_sb = big.tile([128, D], fdt)
            ns_sb = big.tile([128, D], fdt)
            nc.sync.dma_start(nd_sb[:], nd09[:])
            nc.sync.dma_start(ns_sb[:], ns1[:])
            ident = big.tile([64, 64], fdt)
            make_identity(nc, ident[:])

            # ------------------------------------------------ MLP -> h0a
            w1b = big.tile([128, 4, F_HID], bdt)
            for kc in range(4):
                nc.gpsimd.dma_start(w1b[:, kc, :], w1[kc * 128:(kc + 1) * 128, :])
            w2b = big.tile([128, 2, F_OUT], bdt)
            for mh in range(2):
                nc.gpsimd.dma_start(w2b[:, mh, :], w2[mh * 128:(mh + 1) * 128, :])
            b1s = big.tile([128, 2], fdt)
            nc.sync.dma_start(b1s[:, 0:1], b1[0:128, :])
            nc.sync.dma_start(b1s[:, 1:2], b1[128:256, :])
            b2s = big.tile([64, 1], fdt)
            nc.sync.dma_start(b2s[:, :], b2[:, :])

            NT = 512
            tiles = [(i * NT, NT) for i in range(SH // NT)]
            if SH % NT:
                tiles.append((SH - SH % NT, SH % NT))
            with tc.tile_pool(name="mlp", bufs=3) as mp:
                for (r0, nr) in tiles:
                    xtb = mp.tile([128, 4, NT], bdt, tag="xtb")
                    for kc in range(4):
                        nc.sync.dma_start(
                            xtb[:, kc, :nr],
                            xt[kc * 128:(kc + 1) * 128, r0:r0 + nr])
                    h1 = mp.tile([128, 2, NT], bdt, tag="h1")
                    for mh in range(2):
                        ps = pp.tile([128, NT], fdt, tag="ps1")
                        for kc in range(4):
                            nc.tensor.matmul(
                                ps[:, :nr],
                                lhsT=w1b[:, kc, mh * 128:(mh + 1) * 128],
                                rhs=xtb[:, kc, :nr],
                                start=(kc == 0), stop=(kc == 3))
                        nc.scalar.activation(h1[:, mh, :nr], ps[:, :nr],
                                             mybir.ActivationFunctionType.Relu,
                                             bias=b1s[:, mh:mh + 1])
                    ps2 = pp.tile([64, NT], fdt, tag="ps2")
                    for mh in range(2):
                        nc.tensor.matmul(ps2[:, :nr], lhsT=w2b[:, mh, :],
                                         rhs=h1[:, mh, :nr],
                                         start=(mh == 0), stop=(mh == 1))
                    h2 = mp.tile([64, NT], fdt, tag="h2")
                    nc.vector.tensor_scalar_add(h2[:, :nr], ps2[:, :nr],
                                                b2s[:, :])
                    for s in range(nr // 128):
                        pst = pp.tile([128, 64], fdt, tag="pst")
                        nc.tensor.transpose(pst[:], h2[:, s * 128:(s + 1) * 128],
                                            ident[:])
                        j = (r0 + s * 128) // 128
                        nc.scalar.activation(h0a[:, j, :], pst[:],
                                             mybir.ActivationFunctionType.Copy,
                                             scale=ALPHA)
            nc.vector.tensor_scalar_mul(hcur[:], h0a[:], 1.0 / ALPHA)

            # ------------------------------------------------ hops
            with tc.tile_pool(name="hop", bufs=1) as hp, \
                 tc.tile_pool(name="wk", bufs=WK_BUFS) as wk, \
                 tc.tile_pool(name="cgp", bufs=2 if WK_BUFS <= 3 else 1) as cgp:
                for hop in range(K_HOPS):
                    tab = tabs[hop]
                    nc.vector.tensor_tensor(
                        out=sclb[:], in0=hcur[:],
                        in1=ns_sb[:].unsqueeze(-1).to_broadcast([128, D, F_OUT]),
                        op=mybir.AluOpType.mult)
                    nc.sync.dma_start(
                        stag[:, :, 0:F_OUT].rearrange("j p f -> p j f"),
                        sclb[:])
                    if not SKIP_ALLGATHER:
                        if FLAT_AG:
                            ag_in = stag[:].rearrange("j p f -> (j p f)")
                            ag_out = tab[:].rearrange("r f -> (r f)")
                        else:
                            ag_in = stag[:].rearrange("j p f -> (j p) f")
                            ag_out = tab[:]
                        nc.gpsimd.collective_compute(
                            "AllGather", mybir.AluOpType.bypass,
                            replica_groups=[list(range(NCORES))],
                            ins=[ag_in], outs=[ag_out])

                    qn = 0
                    for w in range(NW):
                        # w0 packing is identity: reduce straight into agg.
                        ow = agg if w == 0 else hp.tile([128, D, F_OUT], fdt,
                                                        tag="ow")
                        if SKIP_GATHER or SKIP_REDUCE or (SKIP_W0 and w == 0):
                            nc.vector.memset(ow[:], 0.0)
                        if SKIP_W0 and w == 0:
                            continue
                        for (ww, j0, j1, tot, g16) in chunks_all:
                            if ww != w:
                                continue
                            nidx = tot * 128
                            it = wk.tile([128, (CHUNK_SLOTS + 1) * 8], mybir.dt.int16,
                                         tag="gi")
                            nc.sync.dma_start(it[:, :nidx // 16],
                                              gidx[:, g16:g16 + nidx // 16])
                            msg = wk.tile([128, CHUNK_SLOTS + 1, 2 * F_OUT],
                                          bdt, tag="msg")
                            if not SKIP_GATHER:
                                nc.gpsimd.dma_gather(
                                    msg[:, :tot, :],
                                    tab[WIN_BASES[w] + 32768:
                                        WIN_BASES[w] + 32768 + 32768, :],
                                    it[:, :nidx // 16], nidx, nidx, 2 * F_OUT,
                                    single_packet=SINGLE_PACKET,
                                    queue_num=qn % NQ)
                            qn += 1
                            soff = 0
                            j = j0
                            while j < j1:
                                je = j
                                while je < j1 and P[w, je] == P[w, j]:
                                    je += 1
                                pc = int(P[w, j]); ncol = je - j
                                seg = msg[:, soff:soff + ncol * pc, :F_OUT
                                          ].rearrange(
                                    "p (c q) f -> p c f q", q=pc)
                                if not (SKIP_REDUCE or SKIP_GATHER):
                                    nc.vector.tensor_reduce(
                                        out=ow[:, j:je, :], in_=seg,
                                        axis=mybir.AxisListType.X,
                                        op=mybir.AluOpType.add)
                                soff += ncol * pc
                                j = je
                        if w == 1:
                            nc.sync.dma_start(
                                outw[0][:].rearrange("(v p) f -> p v f", p=128),
                                ow[:])

                    half = SH // 2
                    ch16 = half // 16
                    for hh in range(2):
                        if SKIP_COMBINE:
                            continue
                        cit = wk.tile([128, ch16], mybir.dt.int16, tag="ci")
                        off = hh * ch16
                        nc.sync.dma_start(cit[:], cidx[:, off:off + ch16])
                        cg = cgp.tile([128, D // 2, F_OUT], fdt, tag="cg")
                        nc.gpsimd.dma_gather(
                            cg[:], outw[0][:], cit[:], half, half, F_OUT,
                            single_packet=False, queue_num=qn % NQ)
                        qn += 1
                        dstslc = agg[:, hh * (D // 2):(hh + 1) * (D // 2), :]
                        nc.vector.tensor_add(dstslc, dstslc, cg[:])

                    nc.vector.tensor_tensor(
                        out=hcur[:], in0=agg[:],
                        in1=nd_sb[:].unsqueeze(-1).to_broadcast([128, D, F_OUT]),
                        op=mybir.AluOpType.mult)
                    nc.vector.tensor_add(hcur[:], hcur[:], h0a[:])

            nc.sync.dma_start(hout[:].rearrange("(j p) f -> p j f", p=128),
                              hcur[:])
    nc.finalize()
    return nc


def kernel(features, edge_index, W1, b1, W2, b2):
    core_inputs, struct, perm = preprocess(features, edge_index, W1, b1, W2, b2)
    nc = build_nc(struct)
    from concourse.bass_utils import run_bass_kernel_spmd
    res = run_bass_kernel_spmd(nc, core_inputs, core_ids=list(range(NCORES)))
    out = np.zeros((N, F_OUT), np.float32)
    for c in range(NCORES):
        h = res.results[c]["hout"]                     # [SH, 64] row order
        ids = perm["ids"][c]                           # old ids in row order
        out[ids] = h[:len(ids)]
    return out

